# revision 19
# baseline (speedup 1.0000x reference)
"""Trainium2 Bass kernel for nn_Decoder (GRU vocoder decoder).

Strategy: pure data-parallel over batch (4 sequences per core, 8 cores).
Per core, everything is feature/gate-on-partition:
  phase 0: f0 conv stack + preF0 = f0_cond @ wih0_f0.T + biases (all t),
           h0 = z @ lat_w.T + lat_b
  phase 1: T-step sequential 2-layer GRU scan; weights SBUF-resident bf16
           stationary tiles, h state fp32, psum fp32 accumulation; preF0
           and per-gate biases injected into PSUM via identity-matmuls.
  phase 2: mel projection (big matmul over stored h_b history), postnet
           conv stack as shifted matmuls on padded tiles, residual add.
"""

import numpy as np
import ml_dtypes

import concourse.bass as bass
import concourse.mybir as mybir
from concourse import bacc
from concourse import bass_utils
from concourse.tile import TileContext
from concourse.masks import make_identity

BF16 = np.float16  # fp16: same PE rate as bf16, 8x finer mantissa

LATENT, HIDDEN, NMELS, NLAYERS = 128, 384, 80, 2
F0C, PN = 32, 512
B, T_FULL = 32, 512
NCORES = 8
BL = B // NCORES  # sequences per core
HC = HIDDEN // 128  # h chunks (3)
GC = 2 * HIDDEN // 128  # rz gate chunks (6)

FP32 = mybir.dt.float32
BF16D = mybir.dt.float16
AF = mybir.ActivationFunctionType
ALU = mybir.AluOpType


def build_nc(T: int, dbg: bool = False):
    nc = bacc.Bacc("TRN2", target_bir_lowering=False, debug=False, num_devices=1)

    def din(name, shape, dt=BF16D):
        return nc.dram_tensor(name, list(shape), dt, kind="ExternalInput").ap()

    def dout(name, shape, dt=FP32):
        return nc.dram_tensor(name, list(shape), dt, kind="ExternalOutput").ap()

    # ---- inputs (per-core shards / repacked params) ----
    z_d = din("z", [BL, LATENT], FP32)
    f0_d = din("f0", [BL, T], FP32)

    w0rz_d = din("w0rz", [128, GC, 768])       # contract [h_b; h_a] -> rz gates
    w0ni_d = din("w0ni", [128, HC, HIDDEN])    # contract h_b -> inn0
    w0nh_d = din("w0nh", [128, HC, HIDDEN])    # contract h_a -> hn0
    w1rz_d = din("w1rz", [128, GC, 768])       # contract [h_a; h_b] -> rz gates
    w1ni_d = din("w1ni", [128, HC, HIDDEN])    # contract h_a -> inn1
    w1nh_d = din("w1nh", [128, HC, HIDDEN])    # contract h_b -> hn1
    b0hn_d = din("b0hn", [128, HC, BL])        # bhh0_n broadcast
    b1rz_d = din("b1rz", [128, GC, BL])        # (bih1+bhh1)_rz broadcast
    b1in_d = din("b1in", [128, HC, BL])        # bih1_n broadcast
    b1hn_d = din("b1hn", [128, HC, BL])        # bhh1_n broadcast

    wf0_d = din("wf0", [F0C, 9, 128], FP32)    # wih0[:,384:416].T tiles
    pf0b_d = din("pf0b", [128, 9], FP32)       # rz: bih0+bhh0 ; n: bih0
    latw_d = din("latw", [128, HC, 128])       # lat_w.T tiles
    latb_d = din("latb", [128, HC], FP32)

    c1w_d = din("c1w", [1, 3, F0C], FP32)
    c2w_d = din("c2w", [F0C, 3, F0C], FP32)
    c3w_d = din("c3w", [F0C, 3, F0C], FP32)
    c1b_d = din("c1b", [F0C, 1], FP32)
    c2b_d = din("c2b", [F0C, 1], FP32)
    c3b_d = din("c3b", [F0C, 1], FP32)

    melw_d = din("melw", [128, HC, NMELS])
    melb_d = din("melb", [NMELS, 1], FP32)
    p1w_d = din("p1w", [NMELS, 5, 4, 128])
    p2w_d = din("p2w", [128, 5, 4, 4, 128])
    p3w_d = din("p3w", [128, 5, 4, 4, 128])
    p4w_d = din("p4w", [128, 5, 4, 4, 128])
    p5w_d = din("p5w", [128, 5, 4, NMELS])
    p1b_d = din("p1b", [128, 4], FP32)
    p2b_d = din("p2b", [128, 4], FP32)
    p3b_d = din("p3b", [128, 4], FP32)
    p4b_d = din("p4b", [128, 4], FP32)
    p5b_d = din("p5b", [NMELS, 1], FP32)

    dec_o = dout("dec_o", [BL, NMELS, T])
    post_o = dout("post_o", [BL, NMELS, T])
    if dbg:
        dbg_rz0 = dout("dbg_rz0", [128, GC, BL])
        dbg_n0 = dout("dbg_n0", [128, 2, HC, BL])
        dbg_rzs = dout("dbg_rzs", [128, GC, BL])
        dbg_h0 = dout("dbg_h0", [128, HC, BL])
        dbg_f0c = dout("dbg_f0c", [F0C, BL, T + 2])
        dbg_pf0 = dout("dbg_pf0", [128, 9, BL, T])
        dbg_hbh = dout("dbg_hbh", [128, HC, T + 1, BL])
        dbg_ha1 = dout("dbg_ha1", [128, HC, BL])

    with TileContext(nc) as tc:
        with (
            tc.tile_pool(name="wpool", bufs=1) as wp,
            tc.tile_pool(name="state", bufs=1) as st,
            tc.tile_pool(name="work", bufs=3) as wk,
            tc.tile_pool(name="ppool", bufs=2, space="PSUM") as pp,
        ):
            # ---- load persistent weights ----
            _ld_names = [0]

            def load(d, shape, dt=BF16D):
                _ld_names[0] += 1
                t = wp.tile(list(shape), dt, name=f"w_{_ld_names[0]}")
                nc.sync.dma_start(t[:], d)
                return t

            w0rz = load(w0rz_d, [128, GC, 768])
            w0ni = load(w0ni_d, [128, HC, HIDDEN])
            w0nh = load(w0nh_d, [128, HC, HIDDEN])
            w1rz = load(w1rz_d, [128, GC, 768])
            w1ni = load(w1ni_d, [128, HC, HIDDEN])
            w1nh = load(w1nh_d, [128, HC, HIDDEN])
            b0hn = load(b0hn_d, [128, HC, BL])
            b1rz = load(b1rz_d, [128, GC, BL])
            b1in = load(b1in_d, [128, HC, BL])
            b1hn = load(b1hn_d, [128, HC, BL])
            wf0 = load(wf0_d, [F0C, 9, 128], FP32)
            pf0b = load(pf0b_d, [128, 9], FP32)
            latw = load(latw_d, [128, HC, 128])
            latb = load(latb_d, [128, HC], FP32)
            c1w = load(c1w_d, [1, 3, F0C], FP32)
            c2w = load(c2w_d, [F0C, 3, F0C], FP32)
            c3w = load(c3w_d, [F0C, 3, F0C], FP32)
            c1b = load(c1b_d, [F0C, 1], FP32)
            c2b = load(c2b_d, [F0C, 1], FP32)
            c3b = load(c3b_d, [F0C, 1], FP32)
            melw = load(melw_d, [128, HC, NMELS])
            melb = load(melb_d, [NMELS, 1], FP32)
            p1w = load(p1w_d, [NMELS, 5, 4, 128])
            p2w = load(p2w_d, [128, 5, 4, 4, 128])
            p3w = load(p3w_d, [128, 5, 4, 4, 128])
            p4w = load(p4w_d, [128, 5, 4, 4, 128])
            p5w = load(p5w_d, [128, 5, 4, NMELS])
            p1b = load(p1b_d, [128, 4], FP32)
            p2b = load(p2b_d, [128, 4], FP32)
            p3b = load(p3b_d, [128, 4], FP32)
            p4b = load(p4b_d, [128, 4], FP32)
            p5b = load(p5b_d, [NMELS, 1], FP32)

            ident = wp.tile([128, 128], BF16D)
            make_identity(nc, ident[:])

            # ---- persistent state tensors ----
            # h_b history (bf16): slot 0 = h0, slot t+1 = h_b after step t
            hbh = st.tile([128, HC, T + 1, BL], BF16D)
            # preF0 (bf16): [128, gate chunk (9), seq, t]
            pf0 = st.tile([128, 9, BL, T], BF16D)
            # decoder-side outputs
            dec_sb = st.tile([NMELS, BL, T + 4], FP32)
            dec_bf = st.tile([NMELS, BL, T + 4], BF16D)
            post_sb = st.tile([NMELS, BL, T], FP32)
            prev0 = st.tile([128, HC, BL], BF16D)

            # =========================================================
            # phase 0a: zT, h0
            # =========================================================
            zT = wk.tile([128, BL], FP32, tag="zT")
            nc.sync.dma_start(zT[:], z_d.rearrange("b k -> k b"))
            zT_bf = wk.tile([128, BL], BF16D, tag="zTbf")
            nc.vector.tensor_copy(zT_bf[:], zT[:])

            nc.vector.memset(prev0[:], 0.0)
            nc.vector.tensor_copy(prev0[:, 0, :], zT_bf[:])

            h0ps = pp.tile([128, HC, BL], FP32, tag="ps2")
            for c in range(HC):
                nc.tensor.matmul(h0ps[:, c, :], latw[:, c, :], zT_bf[:],
                                 start=(c == 0), stop=(c == HC - 1))
            ha_f = wk.tile([128, HC, BL], FP32, tag="ha_f")
            hb_f = wk.tile([128, HC, BL], FP32, tag="hb_f")
            for c in range(HC):
                nc.scalar.activation(ha_f[:, c, :], h0ps[:, c, :], AF.Identity,
                                     bias=latb[:, c:c + 1])
            nc.vector.tensor_copy(hb_f[:], ha_f[:])
            ha_b = wk.tile([128, HC, BL], BF16D, tag="ha_b")
            nc.vector.tensor_copy(ha_b[:], ha_f[:])
            nc.vector.tensor_copy(hbh[:, :, 0, :], ha_f[:])
            if dbg:
                nc.sync.dma_start(dbg_h0, ha_f[:])

            # =========================================================
            # phase 0b: f0 conv stack; then preF0
            # =========================================================
            _p0cm = tc.tile_pool(name="p0pool", bufs=1)
            p0 = _p0cm.__enter__()
            f0x = p0.tile([F0C, BL, T + 2], FP32)
            f0y = p0.tile([F0C, BL, T + 2], FP32)
            nc.vector.memset(f0x[:], 0.0)
            nc.vector.memset(f0y[:], 0.0)

            # conv1: 1 -> 32 channels, k=3 pad=1, relu
            for b in range(BL):
                f0raw = p0.tile([1, T + 2], FP32, tag="f0raw", bufs=2)
                nc.vector.memset(f0raw[:], 0.0)
                nc.sync.dma_start(f0raw[:, 1:T + 1], f0_d[b:b + 1, :])
                cps = pp.tile([F0C, T], FP32, tag="ps0")
                for k in range(3):
                    nc.tensor.matmul(cps[:], c1w[:, k, :], f0raw[:, k:k + T],
                                     start=(k == 0), stop=(k == 2))
                nc.scalar.activation(f0x[:, b, 1:T + 1], cps[:], AF.Relu,
                                     bias=c1b[:, 0:1])
            # conv2: 32 -> 32, relu
            for b in range(BL):
                cps = pp.tile([F0C, T], FP32, tag="ps0")
                for k in range(3):
                    nc.tensor.matmul(cps[:], c2w[:, k, :], f0x[:, b, k:k + T],
                                     start=(k == 0), stop=(k == 2))
                nc.scalar.activation(f0y[:, b, 1:T + 1], cps[:], AF.Relu,
                                     bias=c2b[:, 0:1])
            # conv3: 32 -> 32, no relu (back into f0x)
            for b in range(BL):
                cps = pp.tile([F0C, T], FP32, tag="ps0")
                for k in range(3):
                    nc.tensor.matmul(cps[:], c3w[:, k, :], f0y[:, b, k:k + T],
                                     start=(k == 0), stop=(k == 2))
                nc.scalar.activation(f0x[:, b, 1:T + 1], cps[:], AF.Identity,
                                     bias=c3b[:, 0:1])

            if dbg:
                nc.sync.dma_start(dbg_f0c, f0x[:])
            # preF0[g, m, b, t] = sum_i wf0[i, m, g] * f0cond[i, b, t] + bias
            for m in range(9):
                for b in range(BL):
                    pps = pp.tile([128, T], FP32, tag="ps1")
                    nc.tensor.matmul(pps[:], wf0[:, m, :], f0x[:, b, 1:T + 1],
                                     start=True, stop=True)
                    nc.scalar.activation(pf0[:, m, b, :], pps[:], AF.Identity,
                                         bias=pf0b[:, m:m + 1])

            # =========================================================
            # phase 1: the scan
            # =========================================================
            _p0cm.__exit__(None, None, None)
            def gru_cell(rz_ps, n_ps, h_f32, h_bf_new, h_f32_new, extra_bf=None):
                """Gate math. rz_ps: [128,GC,BL] psum; n_ps: [128,2,HC,BL] psum
                (0=inn, 1=hn). h_f32: old state fp32. Writes new fp32 state and
                bf16 copy (h_bf_new AP; optionally extra_bf AP too)."""
                rz_s = wk.tile([128, GC, BL], FP32, tag="rz_s")
                nc.scalar.activation(rz_s[:], rz_ps[:], AF.Sigmoid)
                rhn = wk.tile([128, HC, BL], FP32, tag="rhn")
                nc.vector.tensor_mul(rhn[:], n_ps[:, 1], rz_s[:, 0:HC, :])
                npre = wk.tile([128, HC, BL], FP32, tag="npre")
                nc.vector.tensor_add(npre[:], n_ps[:, 0], rhn[:])
                v = wk.tile([128, HC, BL], FP32, tag="v")
                nc.vector.tensor_scalar(v[:], rz_s[:, HC:GC, :], -1.0, 1.0,
                                        ALU.mult, ALU.add)
                uh = wk.tile([128, HC, BL], FP32, tag="uh")
                nc.vector.tensor_mul(uh[:], rz_s[:, HC:GC, :], h_f32[:])
                n_s = wk.tile([128, HC, BL], FP32, tag="n_s")
                nc.scalar.activation(n_s[:], npre[:], AF.Tanh)
                vn = wk.tile([128, HC, BL], FP32, tag="vn")
                nc.vector.tensor_mul(vn[:], v[:], n_s[:])
                nc.vector.tensor_add(h_f32_new[:], vn[:], uh[:])
                if h_bf_new is not None:
                    nc.vector.tensor_copy(h_bf_new, h_f32_new[:])
                if extra_bf is not None:
                    nc.vector.tensor_copy(extra_bf, h_f32_new[:])

            for t in range(T):
                prev_bf = prev0 if t == 0 else hbh[:, :, t, :]
                hbst_bf = hbh[:, :, t, :]  # h_b state (t-1); slot0 = h0

                # ---- cell 0 ----
                rz0 = pp.tile([128, GC, BL], FP32, tag="ps0")
                n0 = pp.tile([128, 2, HC, BL], FP32, tag="ps1")
                for m in range(GC):
                    for k in range(GC):
                        rhs = prev_bf[:, k, :] if k < HC else ha_b[:, k - HC, :]
                        nc.tensor.matmul(rz0[:, m, :], w0rz[:, k, 128 * m:128 * (m + 1)],
                                         rhs, start=(m == 0 and k == 0), stop=False)
                nc.tensor.matmul(rz0[:], ident[:], pf0[:, 0:GC, :, t],
                                 start=False, stop=True)
                for m in range(HC):
                    for k in range(HC):
                        nc.tensor.matmul(n0[:, 0, m, :], w0ni[:, k, 128 * m:128 * (m + 1)],
                                         prev_bf[:, k, :], start=(m == 0 and k == 0),
                                         stop=False)
                        nc.tensor.matmul(n0[:, 1, m, :], w0nh[:, k, 128 * m:128 * (m + 1)],
                                         ha_b[:, k, :], start=False, stop=False)
                nc.tensor.matmul(n0[:, 0], ident[:], pf0[:, GC:9, :, t],
                                 start=False, stop=True)
                nc.tensor.matmul(n0[:, 1], ident[:], b0hn[:],
                                 start=False, stop=True)
                if dbg and t == 0:
                    _d1 = wk.tile([128, GC, BL], FP32, tag="dbg1")
                    nc.scalar.activation(_d1[:], rz0[:], AF.Copy)
                    nc.sync.dma_start(dbg_rz0, _d1[:])
                    _d2 = wk.tile([128, 2, HC, BL], FP32, tag="dbg2")
                    nc.scalar.activation(_d2[:], n0[:], AF.Copy)
                    nc.sync.dma_start(dbg_n0, _d2[:])

                ha_f_new = wk.tile([128, HC, BL], FP32, tag="ha_f")
                ha_b_new = wk.tile([128, HC, BL], BF16D, tag="ha_b")
                gru_cell(rz0, n0, ha_f, ha_b_new[:], ha_f_new)

                # ---- cell 1 ----
                rz1 = pp.tile([128, GC, BL], FP32, tag="ps2")
                n1 = pp.tile([128, 2, HC, BL], FP32, tag="ps3")
                for m in range(GC):
                    for k in range(GC):
                        rhs = ha_b_new[:, k, :] if k < HC else hbst_bf[:, k - HC, :]
                        nc.tensor.matmul(rz1[:, m, :], w1rz[:, k, 128 * m:128 * (m + 1)],
                                         rhs, start=(m == 0 and k == 0), stop=False)
                nc.tensor.matmul(rz1[:], ident[:], b1rz[:], start=False, stop=True)
                for m in range(HC):
                    for k in range(HC):
                        nc.tensor.matmul(n1[:, 0, m, :], w1ni[:, k, 128 * m:128 * (m + 1)],
                                         ha_b_new[:, k, :], start=(m == 0 and k == 0),
                                         stop=False)
                        nc.tensor.matmul(n1[:, 1, m, :], w1nh[:, k, 128 * m:128 * (m + 1)],
                                         hbst_bf[:, k, :], start=False, stop=False)
                nc.tensor.matmul(n1[:, 0], ident[:], b1in[:], start=False, stop=True)
                nc.tensor.matmul(n1[:, 1], ident[:], b1hn[:], start=False, stop=True)

                hb_f_new = wk.tile([128, HC, BL], FP32, tag="hb_f")
                gru_cell(rz1, n1, hb_f, None, hb_f_new,
                         extra_bf=hbh[:, :, t + 1, :])
                if dbg and t == 0:
                    nc.sync.dma_start(dbg_ha1, ha_f_new[:])
                # note: h_bf_new written directly into hbh slot t+1
                ha_f, ha_b, hb_f = ha_f_new, ha_b_new, hb_f_new

            # =========================================================
            # phase 2: mel projection + postnet
            # =========================================================
            if dbg:
                nc.gpsimd.dma_start(dbg_pf0, pf0[:])
                nc.gpsimd.dma_start(dbg_hbh, hbh[:])
            nc.vector.memset(dec_sb[:], 0.0)
            for b in range(BL):
                mps = pp.tile([NMELS, T], FP32, tag="ps0")
                for c in range(HC):
                    nc.tensor.matmul(mps[:], melw[:, c, :], hbh[:, c, 1:T + 1, b],
                                     start=(c == 0), stop=(c == HC - 1))
                nc.scalar.activation(dec_sb[:, b, 2:T + 2], mps[:], AF.Identity,
                                     bias=melb[:, 0:1])
                nc.sync.dma_start(dec_o[b], dec_sb[:, b, 2:T + 2])
            nc.vector.tensor_copy(dec_bf[:], dec_sb[:])


            def conv5(src_getter, dst, w, bias, n_kc, n_m, func):
                """one seq: y[m] = func(sum_k sum_kc w[kc,k,m].T @ src(kc, k) + b)"""
                for m in range(n_m):
                    cps = pp.tile([128, T], FP32, tag="ps1")
                    first = True
                    for k in range(5):
                        for kc in range(n_kc):
                            nc.tensor.matmul(cps[:], w(kc, k, m), src_getter(kc, k),
                                             start=first, stop=(k == 4 and kc == n_kc - 1))
                            first = False
                    nc.scalar.activation(dst(m), cps[:], func, bias=bias(m))

            for b in range(BL):
                xa = wk.tile([128, 4, T + 4], BF16D, tag="xa", bufs=1)
                xb = wk.tile([128, 4, T + 4], BF16D, tag="xb", bufs=1)
                nc.vector.memset(xa[:], 0.0)
                nc.vector.memset(xb[:], 0.0)
                # p1: 80 -> 512, tanh
                conv5(lambda kc, k: dec_bf[:, b, k:k + T],
                      lambda m: xa[:, m, 2:T + 2],
                      lambda kc, k, m: p1w[:, k, m, :],
                      lambda m: p1b[:, m:m + 1], 1, 4, AF.Tanh)
                # p2..p4: 512 -> 512, tanh
                for w_, b_, src, dst in ((p2w, p2b, xa, xb),
                                         (p3w, p3b, xb, xa),
                                         (p4w, p4b, xa, xb)):
                    conv5(lambda kc, k, s=src: s[:, kc, k:k + T],
                          lambda m, d=dst: d[:, m, 2:T + 2],
                          lambda kc, k, m, ww=w_: ww[:, k, kc, m, :],
                          lambda m, bb=b_: bb[:, m:m + 1], 4, 4, AF.Tanh)
                # p5: 512 -> 80, no tanh; residual add with dec + p5 bias
                cps5 = pp.tile([NMELS, T], FP32, tag="ps2")
                first = True
                for k in range(5):
                    for kc in range(4):
                        nc.tensor.matmul(cps5[:], p5w[:, k, kc, :], xb[:, kc, k:k + T],
                                         start=first, stop=(k == 4 and kc == 3))
                        first = False
                nc.vector.scalar_tensor_tensor(post_sb[:, b, :], cps5[:],
                                               p5b[:, 0:1], dec_sb[:, b, 2:T + 2],
                                               ALU.add, ALU.add)
                nc.sync.dma_start(post_o[b], post_sb[:, b, :])

    nc.compile()
    return nc


# =====================================================================
# host side
# =====================================================================
_NC_CACHE = {}


def _get_nc(T):
    if T not in _NC_CACHE:
        _NC_CACHE[T] = build_nc(T)
    return _NC_CACHE[T]


def _prep_params(p):
    """Repack reference params into device layouts (numpy, host-side)."""
    g = {k: np.asarray(v, np.float32) for k, v in p.items()}
    H = HIDDEN
    out = {}

    def tiles_kxm(w_t):  # [K_total, M_total] -> [128, K_total//128, M_total]
        K, M = w_t.shape
        return np.ascontiguousarray(
            w_t.reshape(K // 128, 128, M).transpose(1, 0, 2)).astype(BF16)

    wih0, whh0 = g["gru0_wih"], g["gru0_whh"]  # [3H, H+F0C], [3H, H]
    wih1, whh1 = g["gru1_wih"], g["gru1_whh"]
    A0 = wih0[:, :H]          # prev part
    WF0 = wih0[:, H:H + F0C]  # f0 part
    # cell0 rz: contract [h_b ; h_a]
    out["w0rz"] = tiles_kxm(np.concatenate([A0[:2 * H].T, whh0[:2 * H].T], 0))
    out["w0ni"] = tiles_kxm(A0[2 * H:].T)
    out["w0nh"] = tiles_kxm(whh0[2 * H:].T)
    # cell1 rz: contract [h_a ; h_b]
    out["w1rz"] = tiles_kxm(np.concatenate([wih1[:2 * H].T, whh1[:2 * H].T], 0))
    out["w1ni"] = tiles_kxm(wih1[2 * H:].T)
    out["w1nh"] = tiles_kxm(whh1[2 * H:].T)

    def bc(vec, nch):  # [nch*128] -> [128, nch, BL] broadcast bf16
        v = vec.reshape(nch, 128).T.astype(np.float32)
        return np.ascontiguousarray(
            np.repeat(v[:, :, None], BL, axis=2)).astype(BF16)

    bih0, bhh0 = g["gru0_bih"], g["gru0_bhh"]
    bih1, bhh1 = g["gru1_bih"], g["gru1_bhh"]
    out["b0hn"] = bc(bhh0[2 * H:], HC)
    out["b1rz"] = bc((bih1 + bhh1)[:2 * H], GC)
    out["b1in"] = bc(bih1[2 * H:], HC)
    out["b1hn"] = bc(bhh1[2 * H:], HC)

    out["wf0"] = np.ascontiguousarray(
        WF0.T.reshape(F0C, 9, 128)).astype(np.float32)
    pf0b = np.concatenate([(bih0 + bhh0)[:2 * H], bih0[2 * H:]])
    out["pf0b"] = np.ascontiguousarray(
        pf0b.reshape(9, 128).T).astype(np.float32)

    out["latw"] = np.ascontiguousarray(
        g["lat_w"].T.reshape(LATENT, HC, 128)).astype(BF16)
    out["latb"] = np.ascontiguousarray(
        g["lat_b"].reshape(HC, 128).T).astype(np.float32)

    out["c1w"] = np.ascontiguousarray(
        g["c1_w"].transpose(1, 2, 0)).astype(np.float32)  # [1,3,32]
    out["c2w"] = np.ascontiguousarray(g["c2_w"].transpose(1, 2, 0)).astype(np.float32)
    out["c3w"] = np.ascontiguousarray(g["c3_w"].transpose(1, 2, 0)).astype(np.float32)
    for n in ("c1", "c2", "c3"):
        out[n + "b"] = g[n + "_b"].reshape(-1, 1).astype(np.float32)

    out["melw"] = np.ascontiguousarray(
        g["mel_w"].T.reshape(HC, 128, NMELS).transpose(1, 0, 2)).astype(BF16)
    out["melb"] = g["mel_b"].reshape(-1, 1).astype(np.float32)

    # postnet: w [O, I, 5]
    out["p1w"] = np.ascontiguousarray(
        g["p1_w"].transpose(1, 2, 0).reshape(NMELS, 5, 4, 128)).astype(BF16)
    for n in ("p2", "p3", "p4"):
        w = g[n + "_w"].transpose(1, 2, 0)  # [I=512, K=5, O=512]
        w = w.reshape(4, 128, 5, 4, 128).transpose(1, 2, 0, 3, 4)
        out[n + "w"] = np.ascontiguousarray(w).astype(BF16)
    w5 = g["p5_w"].transpose(1, 2, 0)  # [512, 5, 80]
    out["p5w"] = np.ascontiguousarray(
        w5.reshape(4, 128, 5, NMELS).transpose(1, 2, 0, 3)).astype(BF16)
    for n in ("p1", "p2", "p3", "p4"):
        out[n + "b"] = np.ascontiguousarray(
            g[n + "_b"].reshape(4, 128).T).astype(np.float32)
    out["p5b"] = g["p5_b"].reshape(-1, 1).astype(np.float32)
    return out


LAST_EXEC_NS = None


def kernel(z, f0, lengths, params, _T=None, _dbg=False, _trace=False):
    z = np.asarray(z, np.float32)
    f0 = np.asarray(f0, np.float32)
    T = f0.shape[1] if _T is None else _T
    dev = _prep_params(params)
    nc = _get_nc(T) if not _dbg else build_nc(T, dbg=True)

    in_maps = []
    for c in range(NCORES):
        m = dict(dev)
        m["z"] = np.ascontiguousarray(z[c * BL:(c + 1) * BL])
        m["f0"] = np.ascontiguousarray(f0[c * BL:(c + 1) * BL, :T])
        in_maps.append(m)

    global LAST_EXEC_NS
    res = bass_utils.run_bass_kernel_spmd(nc, in_maps, core_ids=list(range(NCORES)),
                                          trace=_trace)
    if res.exec_time_ns is not None:
        LAST_EXEC_NS = res.exec_time_ns
    dec = np.concatenate(
        [r["dec_o"].transpose(0, 2, 1) for r in res.results], axis=0)
    post = np.concatenate(
        [r["post_o"].transpose(0, 2, 1) for r in res.results], axis=0)
    if _dbg:
        return dec.astype(np.float32), post.astype(np.float32), res.results
    return dec.astype(np.float32), post.astype(np.float32)
